# revision 21
# baseline (speedup 1.0000x reference)
"""Cox partial-likelihood loss on 8 Trainium2 NeuronCores.

reference:
    theta = hazard_pred.reshape(-1)                 # [n]
    R[i, j] = survtime[j] >= survtime[i]            # risk-set mask
    risk_sum[i] = sum_j exp(theta[j]) * R[i, j]
    loss = -mean((theta - log(risk_sum)) * censor)

Sharding: rows i are split across 8 cores (NL=1024 rows each). Each core
computes its [8192 x 1024] slice of the risk mask in 64 chunks of 128 j's.

v2 design (~30us vs the 52.5us baseline; HW-measured in-situ rates):
  - masks are bf16 with bf16 inputs: DVE tensor_scalar(is_le), single
    ALU op, ~550-590ns/chunk in-kernel (2x DVE mode; the 4x mode shows
    only in single-engine microbenches) vs 723ns for the baseline's
    fp32-input two-ALU-op variant.
  - the mask tile is the matmul STATIONARY operand (8 ldweights+matmul
    per chunk, ~62ns/block in-kernel) and exp(theta) columns are the
    1-wide moving operand, PSUM [128, 8] (partition = i%128, col b =
    i//128). ~495ns/chunk. The mask-as-MOVING form ([1,512] outputs)
    measures ~770ns/chunk in-kernel - back-to-back accumulation into
    one PSUM region serializes on the ~173ns accumulator drain, while
    8 cycling PSUM columns hide it. This drain is also why the old
    baseline was PE-bound at ~50us (not producer-bound as its notes
    assumed).
  - ~1/5 of chunks go to ACT as Sign masks (ACT has no dtype speedup;
    ~1212ns/chunk) to unload DVE. Sign in {-1,0,+1} needs the usual
    encoding corrections, folded cheaply: ACT-chunk matmuls use
    e_half = 0.5*e16 as moving operand, so
       risk = P + 0.5*S_A + 0.5*e_i*[chunk(i) in ACT]
    with S_A = sum of e over ACT-chunk j columns. 0.5*S_A enters as the
    Ln bias (per-partition [128,1] built by a ones-stationary matmul);
    the self-tie row term enters via a [128,1] wv pattern (i mod 64
    depends only on partition). Exp/Sign/Ln/Copy all live in the
    natural_log_exp_and_others ACT table set -> no table switches.
  - survtimes are bf16-rounded on device (st32r) so both compare sides
    round identically: the diagonal j==i gives sign(0)=0 / is_le=1
    exactly. bf16 collision ties off the diagonal contribute +-0.5*e_j
    noise (~1e-3 relative on risk_sum, loss tolerance is 2e-2).

Host side only shards/reorders inputs and sums the 8 partial scalars.
j-index mapping: j = p*64 + c (p = SBUF partition, c = chunk column).
"""

import sys
from contextlib import ExitStack, nullcontext

import numpy as np

try:  # concourse ships with the container toolchain, not on sys.path by default
    import concourse  # noqa: F401
except ImportError:
    sys.path.insert(0, "/opt/trn_rl_repo")

import concourse.bacc as bacc
import concourse.bass as bass
import concourse.tile as tile
from concourse import mybir
from concourse.bass_utils import run_bass_kernel_spmd

DT = mybir.dt
AF = mybir.ActivationFunctionType
N = 8192
CORES = 8
NL = N // CORES       # 1024 local rows per core
NCHUNK = 64           # j-chunks of 128
NBLK = NL // 128      # 8 stationary blocks per chunk

# ACT chunk positions (Sign masks); the rest are DVE is_le chunks.
# Spaced so ACT (slow, in-order) stays ahead of PE's consumption point.
ACT_POS = frozenset(range(4, 62, 4))  # 15 chunks: 4,8,...,60
MASK_BUFS = 16
VARIANT = "full"  # full | pe_* | dve_* | act_only  (bench isolation modes)


def _is_act(c: int) -> bool:
    return c in ACT_POS


_CACHE: dict = {}


def _emit_body(nc, const, masks, psums, tailp, st_all, th_all, st_loc, th_loc,
               cen_loc, wvp, wcol, partial):
    # ---- head: loads ----------------------------------------------------
    # j-major tiles: [p, c] holds index j = p*64 + c
    st_sb = const.tile([128, NCHUNK], DT.float32)
    nc.sync.dma_start(out=st_sb, in_=st_all[:].rearrange("(p c) -> p c", c=NCHUNK))
    th_sb = const.tile([128, NCHUNK], DT.float32)
    nc.sync.dma_start(out=th_sb, in_=th_all[:].rearrange("(p c) -> p c", c=NCHUNK))
    # local survtime broadcast to all partitions (fp32, converted below)
    si32_b = const.tile([128, NL], DT.float32)
    st_loc_row = st_loc[:].rearrange("(o n) -> o n", o=1)
    for q in range(4):
        nc.sync.dma_start(
            out=si32_b[q * 32 : (q + 1) * 32, :],
            in_=st_loc_row.partition_broadcast(32),
        )
    # tail inputs, i-block layout: tile[p, b] = x[b*128 + p]
    thb = tailp.tile([128, NBLK], DT.float32)
    nc.sync.dma_start(out=thb, in_=th_loc[:].rearrange("(b p) -> p b", p=128))
    cenb = tailp.tile([128, NBLK], DT.float32)
    nc.sync.dma_start(out=cenb, in_=cen_loc[:].rearrange("(b p) -> p b", p=128))
    wv_sb = tailp.tile([128, 1], DT.float32)
    nc.sync.dma_start(out=wv_sb, in_=wvp[:].rearrange("(p o) -> p o", o=1))


    # ---- head: converts -------------------------------------------------
    # bf16-rounded survtimes, both sides of every compare
    st16 = const.tile([128, NCHUNK], DT.bfloat16)
    nc.vector.tensor_copy(out=st16, in_=st_sb)
    st32r = const.tile([128, NCHUNK], DT.float32)
    nc.vector.tensor_copy(out=st32r, in_=st16)
    # bf16 copy of the broadcast (free dim = local row i)
    si_b = const.tile([128, NL], DT.bfloat16)
    nc.vector.tensor_copy(out=si_b, in_=si32_b)

    # e = exp(theta), bf16 moving operands
    e32 = const.tile([128, NCHUNK], DT.float32)
    nc.scalar.activation(out=e32, in_=th_sb, func=AF.Exp)
    e16 = const.tile([128, NCHUNK], DT.bfloat16)
    nc.vector.tensor_copy(out=e16, in_=e32)
    e_half = const.tile([128, NCHUNK], DT.bfloat16)
    nc.vector.tensor_scalar(
        out=e_half, in0=e16, scalar1=0.5, scalar2=None,
        op0=mybir.AluOpType.mult,
    )

    # ---- main loop: mask produce + 8 stationary-matmuls per chunk -------
    pt = psums.tile([128, NBLK], DT.float32, tag="pt")
    if VARIANT.startswith("pe_"):
        # premade masks; vary operands selectively to find PE's AP-change tax
        ntile = 8 if VARIANT in ("pe_only", "pe_8tile") else 1
        premade = []
        for t in range(ntile):
            m = masks.tile([128, NL], DT.bfloat16, tag=f"pm{t}")
            nc.vector.tensor_scalar(
                out=m, in0=si_b, scalar1=st32r[:, t : t + 1], scalar2=None,
                op0=mybir.AluOpType.is_le,
            )
            premade.append(m)
        var_blk = VARIANT in ("pe_only", "pe_blk")
        var_ecol = VARIANT in ("pe_only", "pe_ecol")
        var_psum = VARIANT in ("pe_only", "pe_psum")
        for c in range(NCHUNK):
            m = premade[c % ntile]
            mov = e16[:, c : c + 1] if var_ecol else e16[:, 0:1]
            for b in range(NBLK):
                sl = m[:, b * 128 : (b + 1) * 128] if var_blk else m[:, 0:128]
                po = pt[:, b : b + 1] if var_psum else pt[:, 0:1]
                nc.tensor.matmul(
                    po, sl, mov,
                    start=(c == 0 and (var_psum or b == 0)),
                    stop=(c == NCHUNK - 1 and (var_psum or b == NBLK - 1)),
                )
    elif VARIANT == "dve_nopes":
        # masks never consumed by PE (except last chunk): sem-free writes
        for c in range(NCHUNK):
            m = masks.tile([128, NL], DT.bfloat16, tag="md")
            nc.vector.tensor_scalar(
                out=m, in0=si_b, scalar1=st32r[:, c : c + 1], scalar2=None,
                op0=mybir.AluOpType.is_le,
            )
            if c == NCHUNK - 1:
                for b in range(NBLK):
                    nc.tensor.matmul(
                        pt[:, b : b + 1], m[:, b * 128 : (b + 1) * 128],
                        e16[:, c : c + 1], start=True, stop=True,
                    )
    elif VARIANT in ("dve_fix", "dve_stage"):
        for c in range(NCHUNK):
            m = masks.tile([128, NL], DT.bfloat16, tag="md")
            if VARIANT == "dve_fix":
                sc1 = st32r[:, 0:1]
            else:
                stg = masks.tile([128, 1], DT.float32, tag="stg")
                nc.vector.tensor_copy(out=stg, in_=st32r[:, c : c + 1])
                sc1 = stg
            nc.vector.tensor_scalar(
                out=m, in0=si_b, scalar1=sc1, scalar2=None,
                op0=mybir.AluOpType.is_le,
            )
            nc.tensor.matmul(
                pt[:, 0:1], m[:, 0:128], e16[:, c : c + 1],
                start=(c == 0), stop=(c == NCHUNK - 1),
            )
    else:
        for c in range(NCHUNK):
            if VARIANT == "act_only" or (VARIANT == "full" and _is_act(c)):
                m = masks.tile([128, NL], DT.bfloat16, tag="ma")
                nc.scalar.activation(
                    out=m, in_=si_b, func=AF.Sign,
                    bias=st32r[:, c : c + 1], scale=-1.0,
                )
                mov = e_half[:, c : c + 1]
            else:
                m = masks.tile([128, NL], DT.bfloat16, tag="md")
                nc.vector.tensor_scalar(
                    out=m, in0=si_b, scalar1=st32r[:, c : c + 1], scalar2=None,
                    op0=mybir.AluOpType.is_le,
                )
                mov = e16[:, c : c + 1]
            nblk = 1 if VARIANT in ("dve_only", "act_only") else NBLK
            for b in range(nblk):
                nc.tensor.matmul(
                    pt[:, b : b + 1], m[:, b * 128 : (b + 1) * 128], mov,
                    start=(c == 0), stop=(c == NCHUNK - 1),
                )

    # ---- tail -----------------------------------------------------------
    # S_A/2 per-partition bias: wc_b[p,c] = 0.5*[c in ACT], em = e32*wc_b,
    # colsum[p] = sum_c em, psb[m] = sum_p colsum[p]  (ones-stationary)
    wc_b = const.tile([128, NCHUNK], DT.float32)
    nc.sync.dma_start(
        out=wc_b,
        in_=wcol[:].rearrange("(o n) -> o n", o=1).partition_broadcast(128),
    )
    em = tailp.tile([128, NCHUNK], DT.float32)
    nc.vector.tensor_mul(em, e32, wc_b)
    colsum = tailp.tile([128, 1], DT.float32)
    nc.vector.tensor_reduce(
        out=colsum, in_=em, axis=mybir.AxisListType.X, op=mybir.AluOpType.add
    )
    colsum16 = tailp.tile([128, 1], DT.bfloat16)
    nc.vector.tensor_copy(out=colsum16, in_=colsum)
    ones128 = const.tile([128, 128], DT.bfloat16)
    nc.vector.memset(ones128, 1.0)
    psb = psums.tile([128, 1], DT.float32, tag="psb")
    nc.tensor.matmul(psb, ones128, colsum16, start=True, stop=True)
    sa_b = tailp.tile([128, 1], DT.float32)
    nc.vector.tensor_copy(out=sa_b, in_=psb)

    # self-tie correction: corrb[p,b] = exp(thb)*wv[p]
    elb = tailp.tile([128, NBLK], DT.float32)
    nc.scalar.activation(out=elb, in_=thb, func=AF.Exp)
    corrb = tailp.tile([128, NBLK], DT.float32)
    nc.vector.tensor_scalar(
        out=corrb, in0=elb, scalar1=wv_sb, scalar2=None,
        op0=mybir.AluOpType.mult,
    )

    # risk = P + corrb (+ S_A/2 via Ln bias)
    riskb = tailp.tile([128, NBLK], DT.float32)
    nc.vector.tensor_tensor(out=riskb, in0=pt, in1=corrb, op=mybir.AluOpType.add)
    lnb = tailp.tile([128, NBLK], DT.float32)
    nc.scalar.activation(out=lnb, in_=riskb, func=AF.Ln, bias=sa_b, scale=1.0)

    # partial = sum((theta - ln(risk)) * censor) over local rows
    d = tailp.tile([128, NBLK], DT.float32)
    nc.vector.tensor_sub(d, thb, lnb)
    d2 = tailp.tile([128, NBLK], DT.float32)
    nc.vector.tensor_mul(d2, d, cenb)
    red = tailp.tile([128, 1], DT.float32)
    nc.vector.tensor_reduce(
        out=red, in_=d2, axis=mybir.AxisListType.X, op=mybir.AluOpType.add
    )
    ones1 = const.tile([128, 1], DT.float32)
    nc.vector.memset(ones1, 1.0)
    psc = psums.tile([1, 1], DT.float32, tag="psc")
    nc.tensor.matmul(psc, red, ones1, start=True, stop=True)
    res = tailp.tile([1, 1], DT.float32)
    nc.vector.tensor_copy(out=res, in_=psc)
    nc.sync.dma_start(out=partial[:].rearrange("(o n) -> o n", o=1), in_=res)


def _build_nc(reps: int | None = None) -> bass.Bass:
    nc = bacc.Bacc()
    st_all = nc.declare_dram_parameter("st_all", [N], DT.float32, isOutput=False)
    th_all = nc.declare_dram_parameter("th_all", [N], DT.float32, isOutput=False)
    st_loc = nc.declare_dram_parameter("st_loc", [NL], DT.float32, isOutput=False)
    th_loc = nc.declare_dram_parameter("th_loc", [NL], DT.float32, isOutput=False)
    cen_loc = nc.declare_dram_parameter("cen_loc", [NL], DT.float32, isOutput=False)
    wvp = nc.declare_dram_parameter("wvp", [128], DT.float32, isOutput=False)
    wcol = nc.declare_dram_parameter("wcol", [NCHUNK], DT.float32, isOutput=False)
    partial = nc.declare_dram_parameter("partial", [1], DT.float32, isOutput=True)

    with tile.TileContext(nc) as tc, ExitStack() as ctx:
        const = ctx.enter_context(tc.tile_pool(name="const", bufs=1))
        masks = ctx.enter_context(tc.tile_pool(name="masks", bufs=MASK_BUFS))
        psums = ctx.enter_context(tc.tile_pool(name="psums", bufs=1, space="PSUM"))
        tailp = ctx.enter_context(tc.tile_pool(name="tailp", bufs=1))

        loop = (
            tc.For_i(0, reps, 1,
                     hint_engines=(mybir.EngineType.PE, mybir.EngineType.DVE))
            if reps is not None
            else nullcontext()
        )
        with loop:
            _emit_body(nc, const, masks, psums, tailp, st_all, th_all, st_loc,
                       th_loc, cen_loc, wvp, wcol, partial)

    nc.compile()
    return nc


def _get_nc() -> bass.Bass:
    if "nc" not in _CACHE:
        _CACHE["nc"] = _build_nc()
    return _CACHE["nc"]


def _w_patterns():
    act = np.array([0.5 if _is_act(c) else 0.0 for c in range(NCHUNK)],
                   dtype=np.float32)
    wvp = np.tile(act, 128 // NCHUNK).astype(np.float32)  # wv[p] = act[p % 64]
    return wvp, act


def make_in_maps(survtime: np.ndarray, theta: np.ndarray, censor: np.ndarray):
    st = np.ascontiguousarray(survtime, dtype=np.float32)
    th = np.ascontiguousarray(theta, dtype=np.float32).reshape(-1)
    cen = np.ascontiguousarray(censor, dtype=np.float32)
    wvp, wcol = _w_patterns()
    in_maps = []
    for k in range(CORES):
        lo, hi = k * NL, (k + 1) * NL
        in_maps.append(
            {
                "st_all": st,
                "th_all": th,
                "st_loc": st[lo:hi].copy(),
                "th_loc": th[lo:hi].copy(),
                "cen_loc": cen[lo:hi].copy(),
                "wvp": wvp,
                "wcol": wcol,
            }
        )
    return in_maps


def kernel(hazard_pred: np.ndarray, survtime: np.ndarray, censor: np.ndarray):
    nc = _get_nc()
    in_maps = make_in_maps(survtime, hazard_pred, censor)
    out = run_bass_kernel_spmd(nc, in_maps, list(range(CORES)))
    partials = np.array(
        [np.asarray(out.results[k]["partial"]).reshape(-1)[0] for k in range(CORES)],
        dtype=np.float64,
    )
    return np.float32(-partials.sum() / N)
